# revision 5
# baseline (speedup 1.0000x reference)
"""Trainium2 Bass kernel for nn_AnchorDeformAtt (deformable anchor attn).

Sharding: spatial L-shard across 8 cores (core i: pixels [512i, 512(i+1))
for both batches, all heads). Zero collectives; host concatenates shards.

Structure:
  - Gather channels = 8 heads x 16ch; the d=8 table packs, per pixel j,
    the full bilinear 2x2 quad for BOTH channel halves:
    (c: j, j+1, j+64, j+65; c+16: same). ONE ap_gather index per sample
    covers all 4 taps and all 32 channels of a head: 16384 billed indices
    total (HW ap_gather costs ~14-26 ns/index regardless of num_idxs or
    table size, so index count is everything).
  - l-major gather stream (position i = l*16 + p): the "wrapped in 16
    partitions" index layout is then EXACTLY the natural [128=(h,p), l]
    index tile -> no index staging DMAs at all.
  - u = attn * bilinear quad weights staged via PE transposes into DRAM
    [h][l*128 + p*8 + s] (s = chalf x quad), then broadcast to each
    head's 16 channels with fully-contiguous 3-dim DMAs
    ([[LSH*128,8],[0,16],[1,4096]]), double-buffered.
  - Combine: per 2048-idx chunk, stride-8-slot rhs matmuls (128 cols)
    accumulate the out_proj over all (tap, point, chalf) slots;
    contraction covers all 8 heads at once; BN fused in the PSUM copy.
  - Value conv in bf16 (feat + weights host-converted), both channel
    groups from one feat load; quad slots written via ACT + DVE.
  - mem tables rotate through one 64KB/partition SBUF slot (b1's value
    conv lands after b0's last gather releases it).
"""
from contextlib import ExitStack

import ml_dtypes
import numpy as np

import concourse.bass as bass
import concourse.mybir as mybir
import concourse.tile as tile
from concourse import bacc
from concourse.bass_utils import run_bass_kernel_spmd

NH, NP = 8, 16
B, C, H, W = 2, 256, 64, 64
L = H * W            # 4096
NCORES = 8
LSH = L // NCORES    # 512
NLB = LSH // 128
EPS = 1e-6
F32 = mybir.dt.float32
BF16 = mybir.dt.bfloat16
I16 = mybir.dt.int16

_GRAPH_CACHE = {}


def build_v2(stub_gather=False, stub_ubc=False):
    key = ("v2", stub_gather, stub_ubc)
    if key in _GRAPH_CACHE:
        return _GRAPH_CACHE[key]
    DT = BF16
    NIDX = 2048               # indices per ap_gather
    CH = NIDX * 8             # elems per gather chunk (d=8)

    nc = bacc.Bacc("TRN2", target_bir_lowering=False, debug=False,
                   num_devices=NCORES)
    dp = nc.declare_dram_parameter
    feat = dp("feat", [128, B, 2, L], BF16, isOutput=False)
    feat_sh = dp("feat_sh", [128, B, 2, LSH], F32, isOutput=False)
    wv_t = dp("wv_t", [128, 2, 2, 128], BF16, isOutput=False)    # K,cg,kc,M
    bv_p = dp("bv_p", [128, 2], F32, isOutput=False)             # p_new, cg
    woffx_t = dp("woffx_t", [128, 2, 128], F32, isOutput=False)  # K,kc,M
    woffy_t = dp("woffy_t", [128, 2, 128], F32, isOutput=False)
    boffx_p = dp("boffx_p", [128, 1], F32, isOutput=False)
    boffy_p = dp("boffy_p", [128, 1], F32, isOutput=False)
    wszx_t = dp("wszx_t", [128, 2, 8], F32, isOutput=False)
    wszy_t = dp("wszy_t", [128, 2, 8], F32, isOutput=False)
    bszx_p = dp("bszx_p", [8, 1], F32, isOutput=False)
    bszy_p = dp("bszy_p", [8, 1], F32, isOutput=False)
    watt_t = dp("watt_t", [128, 2, 128], F32, isOutput=False)    # K,kc,N
    batt_r = dp("batt_r", [1, 128], F32, isOutput=False)
    ones1 = dp("ones1", [1, 128], F32, isOutput=False)
    ident = dp("ident", [128, 128], F32, isOutput=False)
    wout_t = dp("wout_t", [128, 2, 2, 128], F32, isOutput=False)  # K(cg),cg,oc,M
    bn_s = dp("bn_s", [128, 2], F32, isOutput=False)
    bn_b = dp("bn_b", [128, 2], F32, isOutput=False)
    cenx2 = dp("cenx2", [128, LSH], F32, isOutput=False)
    ceny2 = dp("ceny2", [128, LSH], F32, isOutput=False)
    out_e = dp("out", [B, 2, 128, LSH], F32, isOutput=True)

    # DRAM scratch.
    # ud[b]: [h 8][l*128 + p*8 + s] bf16  (l-major, matches gather out;
    # s = (chalf, quad-tap) with u duplicated across chalf)
    ud_b = [nc.dram_tensor(f"ud{b}", [NH, LSH * 128], DT) for b in range(B)]
    szd = nc.dram_tensor("szd", [B, 2, 8, LSH], F32)

    AP = bass.AP
    Act = mybir.ActivationFunctionType
    Alu = mybir.AluOpType

    with tile.TileContext(nc) as tc, ExitStack() as ctx:
        consts = ctx.enter_context(tc.tile_pool(name="consts", bufs=1))
        featp = ctx.enter_context(tc.tile_pool(name="featp", bufs=2))
        memp = ctx.enter_context(tc.tile_pool(name="memp", bufs=1))
        prep = ctx.enter_context(tc.tile_pool(name="prep", bufs=1))
        gm = ctx.enter_context(tc.tile_pool(name="gm", bufs=1))
        gathp = ctx.enter_context(tc.tile_pool(name="gathp", bufs=2))
        ubcp = ctx.enter_context(tc.tile_pool(name="ubcp", bufs=2))
        outp = ctx.enter_context(tc.tile_pool(name="outp", bufs=1))
        ps_v = ctx.enter_context(tc.tile_pool(name="ps_v", bufs=2, space="PSUM"))
        ps_p = ctx.enter_context(tc.tile_pool(name="ps_p", bufs=2, space="PSUM"))
        ps_o = ctx.enter_context(tc.tile_pool(name="ps_o", bufs=2, space="PSUM"))

        def dmas(out, in_):          # SP queue
            nc.sync.dma_start(out=out, in_=in_)

        def dmaa(out, in_):          # ACT queue
            nc.scalar.dma_start(out=out, in_=in_)

        # ---------------- constants ----------------
        def cload(param, shape, dt=F32):
            t = consts.tile(list(shape), dt, tag=param.name, name=f"c_{param.name}")
            dmas(t[:], param.ap())
            return t
        wv_sb = cload(wv_t, [128, 2, 2, 128], BF16)
        bv_sb = cload(bv_p, [128, 2])
        woffx_sb = cload(woffx_t, [128, 2, 128])
        woffy_sb = cload(woffy_t, [128, 2, 128])
        boffx_sb = cload(boffx_p, [128, 1])
        boffy_sb = cload(boffy_p, [128, 1])
        wszx_sb = cload(wszx_t, [128, 2, 8])
        wszy_sb = cload(wszy_t, [128, 2, 8])
        bszx_sb = cload(bszx_p, [8, 1])
        bszy_sb = cload(bszy_p, [8, 1])
        watt_sb = cload(watt_t, [128, 2, 128])
        batt_sb = cload(batt_r, [1, 128])
        ones_sb = cload(ones1, [1, 128])
        id_sb = cload(ident, [128, 128])
        bns_sb = cload(bn_s, [128, 2])
        bnb_sb = cload(bn_b, [128, 2])
        cenx_sb = cload(cenx2, [128, LSH])
        ceny_sb = cload(ceny2, [128, LSH])
        wout_f32 = gm.tile([128, 2, 2, 128], F32, tag="t1", name="wout_f32")
        dmas(wout_f32[:], wout_t.ap())
        wout_sb = consts.tile([128, 2, 2, 128], DT)
        nc.vector.tensor_copy(out=wout_sb[:], in_=wout_f32[:])

        # mem c-pair quad tables per b: [128 = 8h x 16c, L, 8] bf16, slots
        # (c0: j, j+1, j+64, j+65; c1: same). One tag: b1's table reuses b0's
        # slot after b0's gathers release it.
        mem_t = [None, None]

        # per-b index tiles [128=(h,p), 512 l] i16 — one index per sample
        # (the d=8 quad covers both taps and both chalves).
        fi_b = [gm.tile([128, 512], I16, tag=f"fi{b}", name=f"fi{b}", bufs=1)
                for b in range(B)]

        # ------------- per-b prep convs + grid math + staging -------------
        def prep_b(b):
            fsh_sb = prep.tile([128, 1, 2, LSH], F32, tag="fsh")
            dmas(fsh_sb[:], feat_sh.ap()[:, b:b + 1, :, :])
            offx = prep.tile([128, LSH], F32, tag="offx")
            offy = prep.tile([128, LSH], F32, tag="offy")
            for dst, wsb, bsb in ((offx, woffx_sb, boffx_sb),
                                  (offy, woffy_sb, boffy_sb)):
                ps = ps_p.tile([128, 512], F32, tag="pp")
                for kc in range(2):
                    nc.tensor.matmul(ps[:], wsb[:, kc, :], fsh_sb[:, 0, kc, :],
                                     start=(kc == 0), stop=(kc == 1))
                nc.scalar.activation(out=dst[:], in_=ps[:], func=Act.Sigmoid,
                                     bias=bsb[:], scale=1.0)
            szx_b = prep.tile([128, LSH], F32, tag="szxb")
            szy_b = prep.tile([128, LSH], F32, tag="szyb")
            for k, (wsb, bsb, dstb) in enumerate(
                    ((wszx_sb, bszx_sb, szx_b), (wszy_sb, bszy_sb, szy_b))):
                ps = ps_p.tile([8, 512], F32, tag="pp", name="psz")
                for kc in range(2):
                    nc.tensor.matmul(ps[:], wsb[:, kc, :], fsh_sb[:, 0, kc, :],
                                     start=(kc == 0), stop=(kc == 1))
                szs = gm.tile([8, LSH], F32, tag="szs")
                nc.scalar.activation(out=szs[:], in_=ps[:], func=Act.Sigmoid,
                                     bias=bsb[:], scale=1.0)
                nc.vector.tensor_scalar(out=szs[:], in0=szs[:], scalar1=0.75,
                                        scalar2=0.25, op0=Alu.min, op1=Alu.max)
                dmaa(AP(tensor=szd, offset=(b * 2 + k) * 8 * LSH,
                        ap=[[LSH, 8], [1, LSH]]), szs[:])
                dmaa(dstb[:],
                     AP(tensor=szd, offset=(b * 2 + k) * 8 * LSH,
                        ap=[[LSH, 8], [0, 16], [1, LSH]]))
            # ---- grid math, all tiles [128 (h,p), LSH] ----
            ixy = []
            for k, (off_k, szb, cen) in enumerate(((offx, szx_b, cenx_sb),
                                                   (offy, szy_b, ceny_sb))):
                t1 = gm.tile([128, LSH], F32, tag="t1")
                nc.vector.tensor_scalar(out=t1[:], in0=szb[:], scalar1=-0.5,
                                        scalar2=None, op0=Alu.mult)
                nc.vector.tensor_tensor(out=t1[:], in0=t1[:], in1=cen[:], op=Alu.add)
                g = gm.tile([128, LSH], F32, tag=f"g{k}")
                nc.vector.tensor_tensor(out=g[:], in0=off_k[:], in1=szb[:],
                                        op=Alu.mult)
                nc.vector.tensor_tensor(out=g[:], in0=g[:], in1=t1[:], op=Alu.add)
                nc.vector.tensor_scalar(out=g[:], in0=g[:], scalar1=1.0,
                                        scalar2=0.0, op0=Alu.min, op1=Alu.max)
                nc.vector.tensor_scalar(out=g[:], in0=g[:], scalar1=float(W - 1),
                                        scalar2=None, op0=Alu.mult)
                ixy.append(g)
            x0f, wxy = [], []
            for k in range(2):
                ci = gm.tile([128, LSH], I16, tag="szs", name="ci")
                nc.vector.tensor_copy(out=ci[:], in_=ixy[k][:])
                cf = gm.tile([128, LSH], F32, tag=f"cf{k}")
                nc.vector.tensor_copy(out=cf[:], in_=ci[:])
                msk = gm.tile([128, LSH], F32, tag="t1", name="msk")
                nc.vector.tensor_tensor(out=msk[:], in0=cf[:], in1=ixy[k][:],
                                        op=Alu.is_gt)
                nc.vector.tensor_tensor(out=cf[:], in0=cf[:], in1=msk[:],
                                        op=Alu.subtract)
                w = gm.tile([128, LSH], F32, tag=f"w{k}")
                nc.vector.tensor_tensor(out=w[:], in0=ixy[k][:], in1=cf[:],
                                        op=Alu.subtract)
                x0f.append(cf)
                wxy.append(w)
            f = gm.tile([128, LSH], F32, tag="g1", name="f0")
            nc.vector.tensor_scalar(out=f[:], in0=x0f[1][:], scalar1=float(W),
                                    scalar2=None, op0=Alu.mult)
            nc.vector.tensor_tensor(out=f[:], in0=f[:], in1=x0f[0][:],
                                    op=Alu.add)
            nc.vector.tensor_copy(out=fi_b[b][:], in_=f[:])
            # attn conv (pixel-major) + softmax + transpose to (h,p)-major
            aT = prep.tile([128, LSH], BF16, tag="aT")
            for lb in range(NLB):
                ps = ps_p.tile([128, 128], F32, tag="pp", name="pa")
                for kc in range(2):
                    nc.tensor.matmul(ps[:], fsh_sb[:, 0, kc, lb * 128:(lb + 1) * 128],
                                     watt_sb[:, kc, :], start=(kc == 0), stop=False)
                nc.tensor.matmul(ps[:], ones_sb[:], batt_sb[:],
                                 start=False, stop=True)
                ae = gm.tile([128, 8, 16], F32, tag="ae")
                nc.scalar.activation(out=ae[:], in_=ps[:], func=Act.Exp)
                ssum = gm.tile([128, 8, 1], F32, tag="ssum")
                nc.vector.tensor_reduce(out=ssum[:], in_=ae[:],
                                        axis=mybir.AxisListType.X, op=Alu.add)
                nc.vector.reciprocal(out=ssum[:], in_=ssum[:])
                sap = ssum[:]
                nc.vector.tensor_tensor(
                    out=ae[:], in0=ae[:],
                    in1=AP(tensor=sap.tensor, offset=sap.offset,
                           ap=[sap.ap[0], [1, 8], [0, 16]]),
                    op=Alu.mult)
                pst = ps_p.tile([128, 128], F32, tag="pp", name="pt")
                nc.tensor.transpose(pst[:], ae[:].rearrange("p a b -> p (a b)"),
                                    id_sb[:])
                nc.scalar.activation(out=aT[:, lb * 128:(lb + 1) * 128],
                                     in_=pst[:], func=Act.Copy)

            omx = gm.tile([128, LSH], F32, tag="t1", name="omx")
            nc.vector.tensor_scalar(out=omx[:], in0=wxy[0][:], scalar1=-1.0,
                                    scalar2=1.0, op0=Alu.mult, op1=Alu.add)
            omy = gm.tile([128, LSH], F32, tag="g0", name="omy")
            nc.vector.tensor_scalar(out=omy[:], in0=wxy[1][:], scalar1=-1.0,
                                    scalar2=1.0, op0=Alu.mult, op1=Alu.add)
            ay0 = gm.tile([128, LSH], F32, tag="ay0")
            nc.vector.tensor_tensor(out=ay0[:], in0=aT[:], in1=omy[:], op=Alu.mult)
            ay1 = gm.tile([128, LSH], F32, tag="ay1")
            nc.vector.tensor_tensor(out=ay1[:], in0=aT[:], in1=wxy[1][:],
                                    op=Alu.mult)

            # ---- transpose u factors to [l-local, (lb, (h,p))] and stage ----
            # ayT0/ayT1/omxT/wxT: [128 = l%128, 4 lb, 128 (h,p)] f32
            tps = {}
            for nm, src in (("ayT0", ay0), ("ayT1", ay1),
                            ("omxT", omx), ("wxT", wxy[0])):
                dst = gm.tile([128, NLB, 128], DT, tag=nm)
                for lb in range(NLB):
                    pst = ps_p.tile([128, 128], F32, tag="pp", name=f"tp{nm}{lb}")
                    nc.tensor.transpose(pst[:], src[:, lb * 128:(lb + 1) * 128],
                                        id_sb[:])
                    if lb % 2 == 0:
                        nc.scalar.activation(out=dst[:, lb, :], in_=pst[:],
                                             func=Act.Copy)
                    else:
                        nc.vector.tensor_copy(out=dst[:, lb, :], in_=pst[:])
                tps[nm] = dst
            # upT: [128 = l%128, 4 lb, 1024 = (h,p)*8+s] bf16, pattern
            # s = (ay0*omx, ay0*wx, ay1*omx, ay1*wx) x2 (chalf dup)
            upT = gm.tile([128, NLB, 1024], DT, tag="upT", name=f"upT{b}")
            for lb in range(NLB):
                for sslot, (at, xw) in enumerate(
                        (("ayT0", "omxT"), ("ayT0", "wxT"),
                         ("ayT1", "omxT"), ("ayT1", "wxT")) * 2):
                    o3 = upT[:, lb, :].rearrange("p (a b) -> p a b", b=8)
                    nc.vector.tensor_tensor(out=o3[:, :, sslot],
                                            in0=tps[at][:, lb, :],
                                            in1=tps[xw][:, lb, :],
                                            op=Alu.mult)
                dmas(AP(tensor=ud_b[b], offset=lb * 128 * 128,
                        ap=[[128, 128], [LSH * 128, 8], [1, 128]]),
                     upT[:, lb, :])

        # ------------- value conv per b (both cg from one feat load) -------
        def value_b(b):
            mem_t[b] = memp.tile([128, L, 8], DT, tag="mem", name=f"mem{b}",
                                 bufs=1)
            nc.vector.memset(mem_t[b][:, L - 66:, :], 0.0)
            for n in range(L // 512):
                fts = []
                for kc in range(2):
                    ft = featp.tile([128, 512], BF16, tag="ft")
                    dmas(ft[:], feat.ap()[:, b, kc, n * 512:(n + 1) * 512])
                    fts.append(ft)
                for cg in range(2):
                    ps = ps_v.tile([128, 512], F32, tag="pv")
                    for kc in range(2):
                        nc.tensor.matmul(ps[:], wv_sb[:, cg, kc, :], fts[kc][:],
                                         start=(kc == 0), stop=(kc == 1))
                    # quad slots (j, j+1, j+64, j+65), ACT/DVE alternating
                    for qi, sh in enumerate((0, 1, 64, 65)):
                        ss = 4 * cg + qi
                        if n == 0:
                            o1 = mem_t[b][:, 0:512 - sh, ss]
                            i1 = ps[:, sh:512]
                        else:
                            o1 = mem_t[b][:, n * 512 - sh:(n + 1) * 512 - sh, ss]
                            i1 = ps[:]
                        if qi % 2 == 0:
                            nc.scalar.activation(out=o1, in_=i1,
                                                 func=Act.Identity,
                                                 bias=bv_sb[:, cg:cg + 1],
                                                 scale=1.0)
                        else:
                            nc.vector.tensor_scalar(out=o1, in0=i1,
                                                    scalar1=bv_sb[:, cg:cg + 1],
                                                    scalar2=None, op0=Alu.add)

        # ------------- gather + combine + out per b -------------
        def gather_b(b):
            pso = [ps_o.tile([128, 512], F32, tag=f"po{oc}",
                             name=f"po{oc}_{b}") for oc in range(2)]
            n_acc = 16 * 8              # p * s per (b, oc, lq)
            for lq in range(4):
                g = gathp.tile([128, CH], DT)
                if stub_gather:
                    nc.gpsimd.ap_gather(
                        g[:, 0:128],
                        mem_t[b][:, :, :].rearrange("p a b -> p (a b)"),
                        fi_b[b][:, lq * 128:lq * 128 + 1],
                        channels=128, num_elems=L, d=8, num_idxs=16)
                else:
                    nc.gpsimd.ap_gather(
                        g[:],
                        mem_t[b][:, :, :].rearrange("p a b -> p (a b)"),
                        fi_b[b][:, lq * 128:(lq + 1) * 128],
                        channels=128, num_elems=L, d=8, num_idxs=NIDX)
                for q in range(4):
                    ubc = ubcp.tile([128, CH // 4], DT, tag="ubc",
                                    name=f"ubc{b}{lq}{q}")
                    if not stub_ubc:
                        dma = dmas if q % 2 == 0 else dmaa
                        dma(ubc[:],
                            AP(tensor=ud_b[b],
                               offset=lq * 16384 + q * 4096,
                               ap=[[LSH * 128, 8], [0, 16], [1, CH // 4]]))
                    else:
                        nc.vector.memset(ubc[:, 0:64], 0.0)
                    nc.vector.tensor_tensor(
                        out=g[:, q * (CH // 4):(q + 1) * (CH // 4)],
                        in0=g[:, q * (CH // 4):(q + 1) * (CH // 4)],
                        in1=ubc[:], op=Alu.mult)
                gap = g[:]
                for oc in range(2):
                    cnt = 0
                    for pr in range(16):
                        for ss in range(8):
                            rhs = AP(tensor=gap.tensor,
                                     offset=gap.offset + pr * 8 + ss,
                                     ap=[gap.ap[0], [128, 128]])
                            nc.tensor.matmul(
                                pso[oc][:, lq * 128:(lq + 1) * 128],
                                wout_sb[:, ss // 4, oc, :],
                                rhs, start=(cnt == 0),
                                stop=(cnt == n_acc - 1))
                            cnt += 1
            for oc in range(2):
                o_sb = outp.tile([128, 512], F32, tag="osb")
                nc.scalar.activation(out=o_sb[:], in_=pso[oc][:],
                                     func=Act.Identity,
                                     bias=bnb_sb[:, oc:oc + 1],
                                     scale=bns_sb[:, oc:oc + 1])
                dmaa(AP(tensor=out_e, offset=((b * 2 + oc) * 128) * LSH,
                        ap=[[LSH, 128], [1, LSH]]), o_sb[:])

        # issue order: value+prep b0 -> value+prep b1 -> gathers b0 -> b1
        value_b(0)
        prep_b(0)
        value_b(1)
        prep_b(1)
        gather_b(0)
        gather_b(1)

    nc.compile()
    _GRAPH_CACHE[key] = nc
    return nc


def stage_inputs_v2(inputs, core):
    """Per-core in_map for the v2 graph (all pre-laid-out for plain DMAs)."""
    feat = np.ascontiguousarray(
        np.asarray(inputs['feat_sd'], np.float32).reshape(B, C, L))
    lo = core * LSH
    WvT = np.asarray(inputs['value_proj_w'], np.float32).T.copy()
    WoffT = np.asarray(inputs['anchor_deform_w'], np.float32).T.copy()
    WattT = np.asarray(inputs['anchor_att_w'], np.float32).T.copy()
    WszT = np.asarray(inputs['size_deform_w'], np.float32).T.copy()
    WoutT = np.asarray(inputs['out_proj_w'], np.float32).T.copy()
    boff = np.asarray(inputs['anchor_deform_b'], np.float32)
    bsz = np.asarray(inputs['size_deform_b'], np.float32)
    bv = np.asarray(inputs['value_proj_b'], np.float32)
    cols = (np.arange(W) + 0.5) / (W + EPS)
    rows = (np.arange(H) + 0.5) / (H + EPS)
    cx = np.tile(cols, H)[lo:lo + LSH].astype(np.float32)
    cy = np.repeat(rows, W)[lo:lo + LSH].astype(np.float32)
    fr = feat.reshape(B, 2, 128, L)

    # channel permutation: pass cg row r -> orig channel (r//16)*32+16cg+r%16
    perm = np.zeros((2, 128), np.int64)
    for cg in range(2):
        r = np.arange(128)
        perm[cg] = (r // 16) * 32 + 16 * cg + (r % 16)

    wv = np.zeros((128, 2, 2, 128), np.float32)
    for cg in range(2):
        for kc in range(2):
            wv[:, cg, kc, :] = WvT[kc * 128:(kc + 1) * 128][:, perm[cg]]
    bvp = np.zeros((128, 2), np.float32)
    for cg in range(2):
        bvp[:, cg] = bv[perm[cg]]
    wout = np.zeros((128, 2, 2, 128), np.float32)
    for cg in range(2):
        for oc in range(2):
            wout[:, cg, oc, :] = WoutT[perm[cg]][:, oc * 128:(oc + 1) * 128]

    m = {
        'feat': np.ascontiguousarray(
            fr.transpose(2, 0, 1, 3)).astype(ml_dtypes.bfloat16),
        'feat_sh': np.ascontiguousarray(
            fr[:, :, :, lo:lo + LSH].transpose(2, 0, 1, 3)),
        'wv_t': np.ascontiguousarray(wv).astype(ml_dtypes.bfloat16),
        'bv_p': np.ascontiguousarray(bvp),
        'woffx_t': np.ascontiguousarray(
            WoffT[:, 0::2].reshape(2, 128, 128).transpose(1, 0, 2)),
        'woffy_t': np.ascontiguousarray(
            WoffT[:, 1::2].reshape(2, 128, 128).transpose(1, 0, 2)),
        'boffx_p': np.ascontiguousarray(boff[0::2].reshape(128, 1)),
        'boffy_p': np.ascontiguousarray(boff[1::2].reshape(128, 1)),
        'wszx_t': np.ascontiguousarray(
            WszT[:, 0::2].reshape(2, 128, 8).transpose(1, 0, 2)),
        'wszy_t': np.ascontiguousarray(
            WszT[:, 1::2].reshape(2, 128, 8).transpose(1, 0, 2)),
        'bszx_p': np.ascontiguousarray(bsz[0::2].reshape(8, 1)),
        'bszy_p': np.ascontiguousarray(bsz[1::2].reshape(8, 1)),
        'watt_t': np.ascontiguousarray(
            WattT.reshape(2, 128, 128).transpose(1, 0, 2)),
        'batt_r': np.asarray(inputs['anchor_att_b'], np.float32).reshape(1, 128),
        'ones1': np.ones((1, 128), np.float32),
        'ident': np.eye(128, dtype=np.float32),
        'wout_t': np.ascontiguousarray(wout),
        'bn_s': np.ascontiguousarray(
            (np.asarray(inputs['bn_gamma'], np.float32)
             / np.sqrt(np.float32(1.0 + 1e-5))).reshape(2, 128).T),
        'bn_b': np.ascontiguousarray(
            np.asarray(inputs['bn_beta'], np.float32).reshape(2, 128).T),
        'cenx2': np.ascontiguousarray(np.broadcast_to(cx, (128, LSH))),
        'ceny2': np.ascontiguousarray(np.broadcast_to(cy, (128, LSH))),
    }
    return m


def kernel(**inputs):
    nc = build_v2()
    in_maps = [stage_inputs_v2(inputs, i) for i in range(NCORES)]
    res = run_bass_kernel_spmd(nc, in_maps, core_ids=list(range(NCORES)))
    shards = [res.results[i]['out'].reshape(B, C, LSH) for i in range(NCORES)]
    full = np.concatenate(shards, axis=2).reshape(B, C, H, W)
    return full.astype(np.float32)


# revision 6
# speedup vs baseline: 1.0520x; 1.0520x over previous
"""Trainium2 Bass kernel for nn_AnchorDeformAtt (deformable anchor attn).

Sharding: spatial L-shard across 8 cores (core i: pixels [512i, 512(i+1))
for both batches, all heads). Zero collectives; host concatenates shards.

Structure:
  - Gather channels = 8 heads x 16ch; the d=8 table packs, per pixel j,
    the full bilinear 2x2 quad for BOTH channel halves:
    (c: j, j+1, j+64, j+65; c+16: same). ONE ap_gather index per sample
    covers all 4 taps and all 32 channels of a head: 16384 billed indices
    total (HW ap_gather costs ~14-26 ns/index regardless of num_idxs or
    table size, so index count is everything).
  - l-major gather stream (position i = l*16 + p): the "wrapped in 16
    partitions" index layout is then EXACTLY the natural [128=(h,p), l]
    index tile -> no index staging DMAs at all.
  - u = attn * bilinear quad weights staged via PE transposes into DRAM
    [h][l*128 + p*8 + s] (s = chalf x quad), then broadcast to each
    head's 16 channels with fully-contiguous 3-dim DMAs
    ([[LSH*128,8],[0,16],[1,4096]]), double-buffered.
  - Combine: per 2048-idx chunk, stride-8-slot rhs matmuls (128 cols)
    accumulate the out_proj over all (tap, point, chalf) slots;
    contraction covers all 8 heads at once; BN fused in the PSUM copy.
  - Value conv in bf16 (feat + weights host-converted), both channel
    groups from one feat load; quad slots written via ACT + DVE.
  - mem tables rotate through one 64KB/partition SBUF slot (b1's value
    conv lands after b0's last gather releases it).
  - Dummy scratch-psum matmuls after each chunk's combine keep the PE
    clock at full p-state through the next gather's window (PE otherwise
    idles ~12us/chunk and drops to the 2x-slower MID clock, stretching
    combine latency and eating Pool headroom).
"""
from contextlib import ExitStack

import ml_dtypes
import numpy as np

import concourse.bass as bass
import concourse.mybir as mybir
import concourse.tile as tile
from concourse import bacc
from concourse.bass_utils import run_bass_kernel_spmd

NH, NP = 8, 16
B, C, H, W = 2, 256, 64, 64
L = H * W            # 4096
NCORES = 8
LSH = L // NCORES    # 512
NLB = LSH // 128
EPS = 1e-6
F32 = mybir.dt.float32
BF16 = mybir.dt.bfloat16
I16 = mybir.dt.int16

_GRAPH_CACHE = {}


def build_v2(stub_gather=False, stub_ubc=False):
    key = ("v2", stub_gather, stub_ubc)
    if key in _GRAPH_CACHE:
        return _GRAPH_CACHE[key]
    DT = BF16
    NIDX = 2048               # indices per ap_gather
    CH = NIDX * 8             # elems per gather chunk (d=8)

    nc = bacc.Bacc("TRN2", target_bir_lowering=False, debug=False,
                   num_devices=NCORES)
    dp = nc.declare_dram_parameter
    feat = dp("feat", [128, B, 2, L], BF16, isOutput=False)
    feat_sh = dp("feat_sh", [128, B, 2, LSH], F32, isOutput=False)
    wv_t = dp("wv_t", [128, 2, 2, 128], BF16, isOutput=False)    # K,cg,kc,M
    bv_p = dp("bv_p", [128, 2], F32, isOutput=False)             # p_new, cg
    woffx_t = dp("woffx_t", [128, 2, 128], F32, isOutput=False)  # K,kc,M
    woffy_t = dp("woffy_t", [128, 2, 128], F32, isOutput=False)
    boffx_p = dp("boffx_p", [128, 1], F32, isOutput=False)
    boffy_p = dp("boffy_p", [128, 1], F32, isOutput=False)
    wszx_t = dp("wszx_t", [128, 2, 8], F32, isOutput=False)
    wszy_t = dp("wszy_t", [128, 2, 8], F32, isOutput=False)
    bszx_p = dp("bszx_p", [8, 1], F32, isOutput=False)
    bszy_p = dp("bszy_p", [8, 1], F32, isOutput=False)
    watt_t = dp("watt_t", [128, 2, 128], F32, isOutput=False)    # K,kc,N
    batt_r = dp("batt_r", [1, 128], F32, isOutput=False)
    ones1 = dp("ones1", [1, 128], F32, isOutput=False)
    ident = dp("ident", [128, 128], F32, isOutput=False)
    wout_t = dp("wout_t", [128, 2, 2, 128], F32, isOutput=False)  # K(cg),cg,oc,M
    bn_s = dp("bn_s", [128, 2], F32, isOutput=False)
    bn_b = dp("bn_b", [128, 2], F32, isOutput=False)
    cenx2 = dp("cenx2", [128, LSH], F32, isOutput=False)
    ceny2 = dp("ceny2", [128, LSH], F32, isOutput=False)
    out_e = dp("out", [B, 2, 128, LSH], F32, isOutput=True)

    # DRAM scratch.
    # ud[b]: [h 8][l*128 + p*8 + s] bf16  (l-major, matches gather out;
    # s = (chalf, quad-tap) with u duplicated across chalf)
    ud_b = [nc.dram_tensor(f"ud{b}", [NH, LSH * 128], DT) for b in range(B)]
    szd = nc.dram_tensor("szd", [B, 2, 8, LSH], F32)

    AP = bass.AP
    Act = mybir.ActivationFunctionType
    Alu = mybir.AluOpType

    with tile.TileContext(nc) as tc, ExitStack() as ctx:
        consts = ctx.enter_context(tc.tile_pool(name="consts", bufs=1))
        featp = ctx.enter_context(tc.tile_pool(name="featp", bufs=2))
        memp = ctx.enter_context(tc.tile_pool(name="memp", bufs=1))
        prep = ctx.enter_context(tc.tile_pool(name="prep", bufs=1))
        gm = ctx.enter_context(tc.tile_pool(name="gm", bufs=1))
        gathp = ctx.enter_context(tc.tile_pool(name="gathp", bufs=2))
        ubcp = ctx.enter_context(tc.tile_pool(name="ubcp", bufs=2))
        outp = ctx.enter_context(tc.tile_pool(name="outp", bufs=1))
        ps_v = ctx.enter_context(tc.tile_pool(name="ps_v", bufs=2, space="PSUM"))
        ps_p = ctx.enter_context(tc.tile_pool(name="ps_p", bufs=2, space="PSUM"))
        ps_o = ctx.enter_context(tc.tile_pool(name="ps_o", bufs=2, space="PSUM"))

        def dmas(out, in_):          # SP queue
            nc.sync.dma_start(out=out, in_=in_)

        def dmaa(out, in_):          # ACT queue
            nc.scalar.dma_start(out=out, in_=in_)

        # ---------------- constants ----------------
        def cload(param, shape, dt=F32):
            t = consts.tile(list(shape), dt, tag=param.name, name=f"c_{param.name}")
            dmas(t[:], param.ap())
            return t
        wv_sb = cload(wv_t, [128, 2, 2, 128], BF16)
        bv_sb = cload(bv_p, [128, 2])
        woffx_sb = cload(woffx_t, [128, 2, 128])
        woffy_sb = cload(woffy_t, [128, 2, 128])
        boffx_sb = cload(boffx_p, [128, 1])
        boffy_sb = cload(boffy_p, [128, 1])
        wszx_sb = cload(wszx_t, [128, 2, 8])
        wszy_sb = cload(wszy_t, [128, 2, 8])
        bszx_sb = cload(bszx_p, [8, 1])
        bszy_sb = cload(bszy_p, [8, 1])
        watt_sb = cload(watt_t, [128, 2, 128])
        batt_sb = cload(batt_r, [1, 128])
        ones_sb = cload(ones1, [1, 128])
        id_sb = cload(ident, [128, 128])
        bns_sb = cload(bn_s, [128, 2])
        bnb_sb = cload(bn_b, [128, 2])
        cenx_sb = cload(cenx2, [128, LSH])
        ceny_sb = cload(ceny2, [128, LSH])
        wout_f32 = gm.tile([128, 2, 2, 128], F32, tag="t1", name="wout_f32")
        dmas(wout_f32[:], wout_t.ap())
        wout_sb = consts.tile([128, 2, 2, 128], DT)
        nc.vector.tensor_copy(out=wout_sb[:], in_=wout_f32[:])

        # mem c-pair quad tables per b: [128 = 8h x 16c, L, 8] bf16, slots
        # (c0: j, j+1, j+64, j+65; c1: same). One tag: b1's table reuses b0's
        # slot after b0's gathers release it.
        mem_t = [None, None]

        # per-b index tiles [128=(h,p), 512 l] i16 — one index per sample
        # (the d=8 quad covers both taps and both chalves).
        fi_b = [gm.tile([128, 512], I16, tag=f"fi{b}", name=f"fi{b}", bufs=1)
                for b in range(B)]

        # ------------- per-b prep convs + grid math + staging -------------
        def prep_b(b):
            fsh_sb = prep.tile([128, 1, 2, LSH], F32, tag="fsh")
            dmas(fsh_sb[:], feat_sh.ap()[:, b:b + 1, :, :])
            offx = prep.tile([128, LSH], F32, tag="offx")
            offy = prep.tile([128, LSH], F32, tag="offy")
            for dst, wsb, bsb in ((offx, woffx_sb, boffx_sb),
                                  (offy, woffy_sb, boffy_sb)):
                ps = ps_p.tile([128, 512], F32, tag="pp")
                for kc in range(2):
                    nc.tensor.matmul(ps[:], wsb[:, kc, :], fsh_sb[:, 0, kc, :],
                                     start=(kc == 0), stop=(kc == 1))
                nc.scalar.activation(out=dst[:], in_=ps[:], func=Act.Sigmoid,
                                     bias=bsb[:], scale=1.0)
            szx_b = prep.tile([128, LSH], F32, tag="szxb")
            szy_b = prep.tile([128, LSH], F32, tag="szyb")
            for k, (wsb, bsb, dstb) in enumerate(
                    ((wszx_sb, bszx_sb, szx_b), (wszy_sb, bszy_sb, szy_b))):
                ps = ps_p.tile([8, 512], F32, tag="pp", name="psz")
                for kc in range(2):
                    nc.tensor.matmul(ps[:], wsb[:, kc, :], fsh_sb[:, 0, kc, :],
                                     start=(kc == 0), stop=(kc == 1))
                szs = gm.tile([8, LSH], F32, tag="szs")
                nc.scalar.activation(out=szs[:], in_=ps[:], func=Act.Sigmoid,
                                     bias=bsb[:], scale=1.0)
                nc.vector.tensor_scalar(out=szs[:], in0=szs[:], scalar1=0.75,
                                        scalar2=0.25, op0=Alu.min, op1=Alu.max)
                dmaa(AP(tensor=szd, offset=(b * 2 + k) * 8 * LSH,
                        ap=[[LSH, 8], [1, LSH]]), szs[:])
                dmaa(dstb[:],
                     AP(tensor=szd, offset=(b * 2 + k) * 8 * LSH,
                        ap=[[LSH, 8], [0, 16], [1, LSH]]))
            # ---- grid math, all tiles [128 (h,p), LSH] ----
            ixy = []
            for k, (off_k, szb, cen) in enumerate(((offx, szx_b, cenx_sb),
                                                   (offy, szy_b, ceny_sb))):
                t1 = gm.tile([128, LSH], F32, tag="t1")
                nc.vector.tensor_scalar(out=t1[:], in0=szb[:], scalar1=-0.5,
                                        scalar2=None, op0=Alu.mult)
                nc.vector.tensor_tensor(out=t1[:], in0=t1[:], in1=cen[:], op=Alu.add)
                g = gm.tile([128, LSH], F32, tag=f"g{k}")
                nc.vector.tensor_tensor(out=g[:], in0=off_k[:], in1=szb[:],
                                        op=Alu.mult)
                nc.vector.tensor_tensor(out=g[:], in0=g[:], in1=t1[:], op=Alu.add)
                nc.vector.tensor_scalar(out=g[:], in0=g[:], scalar1=1.0,
                                        scalar2=0.0, op0=Alu.min, op1=Alu.max)
                nc.vector.tensor_scalar(out=g[:], in0=g[:], scalar1=float(W - 1),
                                        scalar2=None, op0=Alu.mult)
                ixy.append(g)
            x0f, wxy = [], []
            for k in range(2):
                ci = gm.tile([128, LSH], I16, tag="szs", name="ci")
                nc.vector.tensor_copy(out=ci[:], in_=ixy[k][:])
                cf = gm.tile([128, LSH], F32, tag=f"cf{k}")
                nc.vector.tensor_copy(out=cf[:], in_=ci[:])
                msk = gm.tile([128, LSH], F32, tag="t1", name="msk")
                nc.vector.tensor_tensor(out=msk[:], in0=cf[:], in1=ixy[k][:],
                                        op=Alu.is_gt)
                nc.vector.tensor_tensor(out=cf[:], in0=cf[:], in1=msk[:],
                                        op=Alu.subtract)
                w = gm.tile([128, LSH], F32, tag=f"w{k}")
                nc.vector.tensor_tensor(out=w[:], in0=ixy[k][:], in1=cf[:],
                                        op=Alu.subtract)
                x0f.append(cf)
                wxy.append(w)
            f = gm.tile([128, LSH], F32, tag="g1", name="f0")
            nc.vector.tensor_scalar(out=f[:], in0=x0f[1][:], scalar1=float(W),
                                    scalar2=None, op0=Alu.mult)
            nc.vector.tensor_tensor(out=f[:], in0=f[:], in1=x0f[0][:],
                                    op=Alu.add)
            nc.vector.tensor_copy(out=fi_b[b][:], in_=f[:])
            # attn conv (pixel-major) + softmax + transpose to (h,p)-major
            aT = prep.tile([128, LSH], BF16, tag="aT")
            for lb in range(NLB):
                ps = ps_p.tile([128, 128], F32, tag="pp", name="pa")
                for kc in range(2):
                    nc.tensor.matmul(ps[:], fsh_sb[:, 0, kc, lb * 128:(lb + 1) * 128],
                                     watt_sb[:, kc, :], start=(kc == 0), stop=False)
                nc.tensor.matmul(ps[:], ones_sb[:], batt_sb[:],
                                 start=False, stop=True)
                ae = gm.tile([128, 8, 16], F32, tag="ae")
                nc.scalar.activation(out=ae[:], in_=ps[:], func=Act.Exp)
                ssum = gm.tile([128, 8, 1], F32, tag="ssum")
                nc.vector.tensor_reduce(out=ssum[:], in_=ae[:],
                                        axis=mybir.AxisListType.X, op=Alu.add)
                nc.vector.reciprocal(out=ssum[:], in_=ssum[:])
                sap = ssum[:]
                nc.vector.tensor_tensor(
                    out=ae[:], in0=ae[:],
                    in1=AP(tensor=sap.tensor, offset=sap.offset,
                           ap=[sap.ap[0], [1, 8], [0, 16]]),
                    op=Alu.mult)
                pst = ps_p.tile([128, 128], F32, tag="pp", name="pt")
                nc.tensor.transpose(pst[:], ae[:].rearrange("p a b -> p (a b)"),
                                    id_sb[:])
                nc.scalar.activation(out=aT[:, lb * 128:(lb + 1) * 128],
                                     in_=pst[:], func=Act.Copy)

            omx = gm.tile([128, LSH], F32, tag="t1", name="omx")
            nc.vector.tensor_scalar(out=omx[:], in0=wxy[0][:], scalar1=-1.0,
                                    scalar2=1.0, op0=Alu.mult, op1=Alu.add)
            omy = gm.tile([128, LSH], F32, tag="g0", name="omy")
            nc.vector.tensor_scalar(out=omy[:], in0=wxy[1][:], scalar1=-1.0,
                                    scalar2=1.0, op0=Alu.mult, op1=Alu.add)
            ay0 = gm.tile([128, LSH], F32, tag="ay0")
            nc.vector.tensor_tensor(out=ay0[:], in0=aT[:], in1=omy[:], op=Alu.mult)
            ay1 = gm.tile([128, LSH], F32, tag="ay1")
            nc.vector.tensor_tensor(out=ay1[:], in0=aT[:], in1=wxy[1][:],
                                    op=Alu.mult)

            # ---- transpose u factors to [l-local, (lb, (h,p))] and stage ----
            # ayT0/ayT1/omxT/wxT: [128 = l%128, 4 lb, 128 (h,p)] f32
            tps = {}
            for nm, src in (("ayT0", ay0), ("ayT1", ay1),
                            ("omxT", omx), ("wxT", wxy[0])):
                dst = gm.tile([128, NLB, 128], DT, tag=nm)
                for lb in range(NLB):
                    pst = ps_p.tile([128, 128], F32, tag="pp", name=f"tp{nm}{lb}")
                    nc.tensor.transpose(pst[:], src[:, lb * 128:(lb + 1) * 128],
                                        id_sb[:])
                    if lb % 2 == 0:
                        nc.scalar.activation(out=dst[:, lb, :], in_=pst[:],
                                             func=Act.Copy)
                    else:
                        nc.vector.tensor_copy(out=dst[:, lb, :], in_=pst[:])
                tps[nm] = dst
            # upT: [128 = l%128, 4 lb, 1024 = (h,p)*8+s] bf16, pattern
            # s = (ay0*omx, ay0*wx, ay1*omx, ay1*wx) x2 (chalf dup)
            upT = gm.tile([128, NLB, 1024], DT, tag="upT", name=f"upT{b}")
            for lb in range(NLB):
                for sslot, (at, xw) in enumerate(
                        (("ayT0", "omxT"), ("ayT0", "wxT"),
                         ("ayT1", "omxT"), ("ayT1", "wxT")) * 2):
                    o3 = upT[:, lb, :].rearrange("p (a b) -> p a b", b=8)
                    nc.vector.tensor_tensor(out=o3[:, :, sslot],
                                            in0=tps[at][:, lb, :],
                                            in1=tps[xw][:, lb, :],
                                            op=Alu.mult)
                dmas(AP(tensor=ud_b[b], offset=lb * 128 * 128,
                        ap=[[128, 128], [LSH * 128, 8], [1, 128]]),
                     upT[:, lb, :])

        # ------------- value conv per b (both cg from one feat load) -------
        def value_b(b):
            mem_t[b] = memp.tile([128, L, 8], DT, tag="mem", name=f"mem{b}",
                                 bufs=1)
            nc.vector.memset(mem_t[b][:, L - 66:, :], 0.0)
            for n in range(L // 512):
                fts = []
                for kc in range(2):
                    ft = featp.tile([128, 512], BF16, tag="ft")
                    dmas(ft[:], feat.ap()[:, b, kc, n * 512:(n + 1) * 512])
                    fts.append(ft)
                for cg in range(2):
                    ps = ps_v.tile([128, 512], F32, tag="pv")
                    for kc in range(2):
                        nc.tensor.matmul(ps[:], wv_sb[:, cg, kc, :], fts[kc][:],
                                         start=(kc == 0), stop=(kc == 1))
                    # quad slots (j, j+1, j+64, j+65), ACT/DVE alternating
                    for qi, sh in enumerate((0, 1, 64, 65)):
                        ss = 4 * cg + qi
                        if n == 0:
                            o1 = mem_t[b][:, 0:512 - sh, ss]
                            i1 = ps[:, sh:512]
                        else:
                            o1 = mem_t[b][:, n * 512 - sh:(n + 1) * 512 - sh, ss]
                            i1 = ps[:]
                        if qi % 2 == 0:
                            nc.scalar.activation(out=o1, in_=i1,
                                                 func=Act.Identity,
                                                 bias=bv_sb[:, cg:cg + 1],
                                                 scale=1.0)
                        else:
                            nc.vector.tensor_scalar(out=o1, in0=i1,
                                                    scalar1=bv_sb[:, cg:cg + 1],
                                                    scalar2=None, op0=Alu.add)

        # ------------- gather + combine + out per b -------------
        def gather_b(b):
            pso = [ps_o.tile([128, 512], F32, tag=f"po{oc}",
                             name=f"po{oc}_{b}") for oc in range(2)]
            n_acc = 16 * 8              # p * s per (b, oc, lq)
            for lq in range(4):
                g = gathp.tile([128, CH], DT)
                if stub_gather:
                    nc.gpsimd.ap_gather(
                        g[:, 0:128],
                        mem_t[b][:, :, :].rearrange("p a b -> p (a b)"),
                        fi_b[b][:, lq * 128:lq * 128 + 1],
                        channels=128, num_elems=L, d=8, num_idxs=16)
                else:
                    nc.gpsimd.ap_gather(
                        g[:],
                        mem_t[b][:, :, :].rearrange("p a b -> p (a b)"),
                        fi_b[b][:, lq * 128:(lq + 1) * 128],
                        channels=128, num_elems=L, d=8, num_idxs=NIDX)
                for q in range(4):
                    ubc = ubcp.tile([128, CH // 4], DT, tag="ubc",
                                    name=f"ubc{b}{lq}{q}")
                    if not stub_ubc:
                        dma = dmas if q % 2 == 0 else dmaa
                        dma(ubc[:],
                            AP(tensor=ud_b[b],
                               offset=lq * 16384 + q * 4096,
                               ap=[[LSH * 128, 8], [0, 16], [1, CH // 4]]))
                    else:
                        nc.vector.memset(ubc[:, 0:64], 0.0)
                    nc.vector.tensor_tensor(
                        out=g[:, q * (CH // 4):(q + 1) * (CH // 4)],
                        in0=g[:, q * (CH // 4):(q + 1) * (CH // 4)],
                        in1=ubc[:], op=Alu.mult)
                gap = g[:]
                for oc in range(2):
                    cnt = 0
                    for pr in range(16):
                        for ss in range(8):
                            rhs = AP(tensor=gap.tensor,
                                     offset=gap.offset + pr * 8 + ss,
                                     ap=[gap.ap[0], [128, 128]])
                            nc.tensor.matmul(
                                pso[oc][:, lq * 128:(lq + 1) * 128],
                                wout_sb[:, ss // 4, oc, :],
                                rhs, start=(cnt == 0),
                                stop=(cnt == n_acc - 1))
                            cnt += 1
                # p-state filler: keep the PE clock ramped through the next
                # gather's window; writes scratch psum, reads only resident
                # consts, so it never delays Pool or the real dataflow.
                if not (b == 1 and lq == 3):
                    for dj in range(28):
                        dps = ps_p.tile([128, 512], F32, tag="pp",
                                        name=f"dum{b}{lq}{dj}")
                        nc.tensor.matmul(
                            dps[:], wout_sb[:, 0, 0, :],
                            wv_sb[:].rearrange("p a b c -> p (a b c)")[:, 0:512],
                            start=True, stop=True)
            for oc in range(2):
                o_sb = outp.tile([128, 512], F32, tag="osb")
                nc.scalar.activation(out=o_sb[:], in_=pso[oc][:],
                                     func=Act.Identity,
                                     bias=bnb_sb[:, oc:oc + 1],
                                     scale=bns_sb[:, oc:oc + 1])
                dmaa(AP(tensor=out_e, offset=((b * 2 + oc) * 128) * LSH,
                        ap=[[LSH, 128], [1, LSH]]), o_sb[:])

        # issue order: value+prep b0 -> value+prep b1 -> gathers b0 -> b1
        value_b(0)
        prep_b(0)
        value_b(1)
        prep_b(1)
        gather_b(0)
        gather_b(1)

    nc.compile()
    _GRAPH_CACHE[key] = nc
    return nc


def stage_inputs_v2(inputs, core):
    """Per-core in_map for the v2 graph (all pre-laid-out for plain DMAs)."""
    feat = np.ascontiguousarray(
        np.asarray(inputs['feat_sd'], np.float32).reshape(B, C, L))
    lo = core * LSH
    WvT = np.asarray(inputs['value_proj_w'], np.float32).T.copy()
    WoffT = np.asarray(inputs['anchor_deform_w'], np.float32).T.copy()
    WattT = np.asarray(inputs['anchor_att_w'], np.float32).T.copy()
    WszT = np.asarray(inputs['size_deform_w'], np.float32).T.copy()
    WoutT = np.asarray(inputs['out_proj_w'], np.float32).T.copy()
    boff = np.asarray(inputs['anchor_deform_b'], np.float32)
    bsz = np.asarray(inputs['size_deform_b'], np.float32)
    bv = np.asarray(inputs['value_proj_b'], np.float32)
    cols = (np.arange(W) + 0.5) / (W + EPS)
    rows = (np.arange(H) + 0.5) / (H + EPS)
    cx = np.tile(cols, H)[lo:lo + LSH].astype(np.float32)
    cy = np.repeat(rows, W)[lo:lo + LSH].astype(np.float32)
    fr = feat.reshape(B, 2, 128, L)

    # channel permutation: pass cg row r -> orig channel (r//16)*32+16cg+r%16
    perm = np.zeros((2, 128), np.int64)
    for cg in range(2):
        r = np.arange(128)
        perm[cg] = (r // 16) * 32 + 16 * cg + (r % 16)

    wv = np.zeros((128, 2, 2, 128), np.float32)
    for cg in range(2):
        for kc in range(2):
            wv[:, cg, kc, :] = WvT[kc * 128:(kc + 1) * 128][:, perm[cg]]
    bvp = np.zeros((128, 2), np.float32)
    for cg in range(2):
        bvp[:, cg] = bv[perm[cg]]
    wout = np.zeros((128, 2, 2, 128), np.float32)
    for cg in range(2):
        for oc in range(2):
            wout[:, cg, oc, :] = WoutT[perm[cg]][:, oc * 128:(oc + 1) * 128]

    m = {
        'feat': np.ascontiguousarray(
            fr.transpose(2, 0, 1, 3)).astype(ml_dtypes.bfloat16),
        'feat_sh': np.ascontiguousarray(
            fr[:, :, :, lo:lo + LSH].transpose(2, 0, 1, 3)),
        'wv_t': np.ascontiguousarray(wv).astype(ml_dtypes.bfloat16),
        'bv_p': np.ascontiguousarray(bvp),
        'woffx_t': np.ascontiguousarray(
            WoffT[:, 0::2].reshape(2, 128, 128).transpose(1, 0, 2)),
        'woffy_t': np.ascontiguousarray(
            WoffT[:, 1::2].reshape(2, 128, 128).transpose(1, 0, 2)),
        'boffx_p': np.ascontiguousarray(boff[0::2].reshape(128, 1)),
        'boffy_p': np.ascontiguousarray(boff[1::2].reshape(128, 1)),
        'wszx_t': np.ascontiguousarray(
            WszT[:, 0::2].reshape(2, 128, 8).transpose(1, 0, 2)),
        'wszy_t': np.ascontiguousarray(
            WszT[:, 1::2].reshape(2, 128, 8).transpose(1, 0, 2)),
        'bszx_p': np.ascontiguousarray(bsz[0::2].reshape(8, 1)),
        'bszy_p': np.ascontiguousarray(bsz[1::2].reshape(8, 1)),
        'watt_t': np.ascontiguousarray(
            WattT.reshape(2, 128, 128).transpose(1, 0, 2)),
        'batt_r': np.asarray(inputs['anchor_att_b'], np.float32).reshape(1, 128),
        'ones1': np.ones((1, 128), np.float32),
        'ident': np.eye(128, dtype=np.float32),
        'wout_t': np.ascontiguousarray(wout),
        'bn_s': np.ascontiguousarray(
            (np.asarray(inputs['bn_gamma'], np.float32)
             / np.sqrt(np.float32(1.0 + 1e-5))).reshape(2, 128).T),
        'bn_b': np.ascontiguousarray(
            np.asarray(inputs['bn_beta'], np.float32).reshape(2, 128).T),
        'cenx2': np.ascontiguousarray(np.broadcast_to(cx, (128, LSH))),
        'ceny2': np.ascontiguousarray(np.broadcast_to(cy, (128, LSH))),
    }
    return m


def kernel(**inputs):
    nc = build_v2()
    in_maps = [stage_inputs_v2(inputs, i) for i in range(NCORES)]
    res = run_bass_kernel_spmd(nc, in_maps, core_ids=list(range(NCORES)))
    shards = [res.results[i]['out'].reshape(B, C, LSH) for i in range(NCORES)]
    full = np.concatenate(shards, axis=2).reshape(B, C, H, W)
    return full.astype(np.float32)
